# revision 19
# baseline (speedup 1.0000x reference)
"""Trainium2 Bass kernel for per-token multi-head self-attention.

Computation (per token t):
  q,k,v = x @ W{q,k,v}.T ; scores = (q_t k_t^T)/sqrt(128) over heads [16x16]
  out_t = softmax(scores) @ v_t ; y = out @ Wo.T

Sharding: data-parallel over the 16384 tokens -> 8 cores x 2048 tokens.

Fully-fused single-pass structure, all matmul operands in bf16 (fp32 PSUM
accumulation; CPU-simulated pipeline rel err ~4.4e-3 vs the 2e-2 gate):
  - One stream of "GEMM units" (QKV projection tiles and Wo output tiles)
    is interleaved ("pumped") between the small attention-middle ops so the
    PE never idles on the middle's cross-engine dependency chains.
  - qkv never round-trips through DRAM: QKV units for chunk c+1 run (as
    pump filler) during the attention middle of chunk c, writing SBUF
    double buffers.
  - Attention middle processes 8 tokens per group: per-token 16x16 score
    matmuls (4 PE column-groups x 2 rounds) -> one exp ACT -> block-diag
    [128,128] attn matrix (copies split across gpsimd/vector/scalar) ->
    one AV matmul against the PE-transposed V block with a ones column
    producing the softmax normalizer -> per-partition 1/Z scale -> one
    PE-transpose back to feature-major layout for the Wo GEMM.
Weights are host-side pre-tiled so every weight-tile DMA reads 2-4KB
contiguous runs.
"""
import math
from contextlib import ExitStack

import numpy as np

NCORES = 8
E = 2048          # hidden
NH = 16           # heads
HD = 128          # head dim
TPC = 2048        # tokens per core
TC = 512          # token chunk
P = 128
NE = E // P       # 16 contraction tiles
NO = E // P       # 16 output tiles
NCH = TPC // TC   # 4 chunks
NG = TC // 8      # 64 8-token groups per chunk
SC = 1.0 / math.sqrt(HD)

_cached = {}


_dbg = {}


def _build_program():
    import concourse.bass as bass
    import concourse.tile as tile
    from concourse import bacc, mybir
    from concourse.masks import make_identity

    f32 = mybir.dt.float32
    bf16 = mybir.dt.bfloat16

    nc = bacc.Bacc("TRN2", target_bir_lowering=False, debug=False)

    xT_d = nc.dram_tensor("xT", [E, TPC], bf16, kind="ExternalInput").ap()
    # pre-tiled weights: row oi*128+p, col e*128+o  (p = input-feature within
    # e-slice for QKV; for Wo: p = head-dim within head h, col h*128+o)
    Wq_d = nc.dram_tensor("Wq", [E, E], bf16, kind="ExternalInput").ap()
    Wk_d = nc.dram_tensor("Wk", [E, E], bf16, kind="ExternalInput").ap()
    Wv_d = nc.dram_tensor("Wv", [E, E], bf16, kind="ExternalInput").ap()
    Wo_d = nc.dram_tensor("Wo", [E, E], bf16, kind="ExternalInput").ap()
    yT_d = nc.dram_tensor("yT", [E, TPC], f32, kind="ExternalOutput").ap()

    with tile.TileContext(nc) as tc, ExitStack() as ctx:
        glob = ctx.enter_context(tc.tile_pool(name="glob", bufs=1))
        ident = glob.tile([P, P], bf16)
        make_identity(nc, ident)

        xp = ctx.enter_context(tc.tile_pool(name="xp", bufs=1))
        qkvp = ctx.enter_context(tc.tile_pool(name="qkvp", bufs=1))
        aotp = ctx.enter_context(tc.tile_pool(name="aotp", bufs=1))
        v2p = ctx.enter_context(tc.tile_pool(name="v2p", bufs=1))
        bdp = ctx.enter_context(tc.tile_pool(name="bdp", bufs=1))
        vgp = ctx.enter_context(tc.tile_pool(name="vgp", bufs=1))
        wp = ctx.enter_context(tc.tile_pool(name="wp", bufs=4))
        esp = ctx.enter_context(tc.tile_pool(name="esp", bufs=3))
        aop = ctx.enter_context(tc.tile_pool(name="aop", bufs=3))
        ivp = ctx.enter_context(tc.tile_pool(name="ivp", bufs=3))
        ysp = ctx.enter_context(tc.tile_pool(name="ysp", bufs=3))
        psG = ctx.enter_context(tc.tile_pool(name="psG", bufs=3, space="PSUM"))
        psS = ctx.enter_context(tc.tile_pool(name="psS", bufs=1, space="PSUM"))
        psM = ctx.enter_context(tc.tile_pool(name="psM", bufs=4, space="PSUM"))

        # persistent double buffers
        xb = [xp.tile([P, NE, TC], bf16, tag=f"x{i}", name=f"x{i}")
              for i in range(2)]
        qkv = [[qkvp.tile([P, NO, TC], bf16, tag=f"qkv{m}_{i}",
                          name=f"qkv{m}_{i}")
                for i in range(2)] for m in range(3)]
        aoT = [aotp.tile([P, NH, TC], bf16, tag=f"aoT{i}", name=f"aoT{i}")
               for i in range(2)]
        v2 = []
        for i in range(2):
            t = v2p.tile([P, 64, 32], bf16, tag=f"v2_{i}", name=f"v2_{i}")
            nc.vector.memset(t, 0.0)
            v2.append(t)
        NBD = 4
        bds = []
        for i in range(NBD):
            t = bdp.tile([P, 280], bf16, tag=f"bd{i}", name=f"bd{i}")
            nc.vector.memset(t, 0.0)
            bds.append(t)
        NVG = 8
        vgs = []
        for i in range(NVG):
            t = vgp.tile([P, HD + 1], bf16, tag=f"vg{i}", name=f"vg{i}")
            nc.vector.memset(t, 0.0)
            nc.vector.memset(t[:, HD:HD + 1], 1.0)
            vgs.append(t)

        wmats = [Wq_d, Wk_d, Wv_d]

        def load_x(c):
            for e in range(NE):
                nc.sync.dma_start(
                    out=xb[c % 2][:, e, :],
                    in_=xT_d[e * P:(e + 1) * P, c * TC:(c + 1) * TC])

        # ---------------- GEMM unit machinery ----------------
        # Each unit: (prefetch_fn -> returns w tile, gen_fn(w) yields per MM)
        def qkv_unit(c, oi, m):
            def pre():
                wt = wp.tile([P, NE, P], bf16, tag="w", name="w")
                wf = wt.rearrange("p e o -> p (e o)")
                nc.sync.dma_start(out=wf[:, 0:E // 2],
                                  in_=wmats[m][oi * P:(oi + 1) * P, 0:E // 2])
                nc.sync.dma_start(out=wf[:, E // 2:E],
                                  in_=wmats[m][oi * P:(oi + 1) * P, E // 2:E])
                return wt

            def gen(wt):
                acc = psG.tile([P, TC], f32, tag="acc", name="acc")
                for e in range(NE):
                    nc.tensor.matmul(acc, wt[:, e, :], xb[c % 2][:, e, :],
                                     start=(e == 0), stop=(e == NE - 1))
                    yield
                nc.scalar.activation(
                    out=qkv[m][c % 2][:, oi, :], in_=acc,
                    func=mybir.ActivationFunctionType.Copy)

            return pre, gen

        def wo_unit(c, oi):
            def pre():
                wt = wp.tile([P, NH, P], bf16, tag="w", name="w")
                wf = wt.rearrange("p h o -> p (h o)")
                nc.sync.dma_start(out=wf[:, 0:E // 2],
                                  in_=Wo_d[oi * P:(oi + 1) * P, 0:E // 2])
                nc.sync.dma_start(out=wf[:, E // 2:E],
                                  in_=Wo_d[oi * P:(oi + 1) * P, E // 2:E])
                return wt

            def gen(wt):
                yp = psG.tile([P, TC], f32, tag="acc", name="yp")
                for h in range(NH):
                    nc.tensor.matmul(yp, wt[:, h, :], aoT[c % 2][:, h, :],
                                     start=(h == 0), stop=(h == NH - 1))
                    yield
                ys = ysp.tile([P, TC], f32, tag="ys", name="ys")
                nc.vector.tensor_copy(ys, yp)
                nc.sync.dma_start(
                    out=yT_d[oi * P:(oi + 1) * P, c * TC:(c + 1) * TC],
                    in_=ys)

            return pre, gen

        pend = []          # [pre, gen] not yet prefetched
        active = []        # generators with w already fetched
        LOOKAHEAD = 4

        def refill():
            while pend and len(active) < LOOKAHEAD:
                pre, gen = pend.pop(0)
                active.append(gen(pre()))

        def pump(n):
            refill()
            while n > 0 and active:
                g = active[0]
                try:
                    next(g)
                    n -= 1
                except StopIteration:
                    active.pop(0)
                    refill()

        def pump_all():
            refill()
            while active:
                try:
                    next(active[0])
                except StopIteration:
                    active.pop(0)
                    refill()

        # ---------------- attention middle ----------------
        def relayout(c, sub, half):
            t0 = sub * 64 + 32 * half
            nc.gpsimd.tensor_copy(
                v2[sub % 2][:, 32 * half:32 * (half + 1), 0:NH],
                qkv[2][c % 2][:, :, t0:t0 + 32]
                .rearrange("p g t -> p t g"))

        state = {"gi": 0, "prev": None}

        def phase_a(c, s):
            sub = s // 8
            if s % 8 == 2 and sub + 1 < 8:
                relayout(c, sub + 1, 0)
            if s % 8 == 5 and sub + 1 < 8:
                relayout(c, sub + 1, 1)
            q_sb, k_sb = qkv[0][c % 2], qkv[1][c % 2]
            sc = psS.tile([P, 32], f32, tag="sc", name="sc")
            t0 = s * 8
            for j in range(4):
                for half in range(2):
                    t = t0 + 4 * half + j
                    nc.tensor.matmul(
                        sc[32 * j:32 * j + NH, 16 * half:16 * half + 16],
                        k_sb[:, :, t], q_sb[:, :, t],
                        start=True, stop=True,
                        tile_position=(0, 32 * j))
            es = esp.tile([P, 32], bf16, tag="es", name="es")
            nc.scalar.activation(out=es, in_=sc,
                                 func=mybir.ActivationFunctionType.Exp,
                                 scale=SC)
            gi = state["gi"]
            # one wide block-diag tile: col 140*hf + 8h + j, rows 32j+g
            bd = bds[gi % NBD]
            bdv = bd.rearrange("p (hf q) -> p hf q", hf=2)
            esv = es.rearrange("p (hf g) -> p hf g", hf=2)
            for j in range(4):
                dst = bdv[32 * j:32 * j + 16, :, j:j + 121:8]
                srcv = esv[32 * j:32 * j + 16, :, :]
                if j in (0, 3):
                    nc.gpsimd.tensor_copy(dst, srcv)
                elif j == 1:
                    nc.scalar.activation(
                        out=dst, in_=srcv,
                        func=mybir.ActivationFunctionType.Copy)
                else:
                    nc.vector.tensor_copy(dst, srcv)
            for half in range(2):
                # V block transpose via DMA XBAR: [128 d, 4t*32] -> [(32t+g), d]
                vg = vgs[(2 * gi + half) % NVG]
                nc.sync.dma_start_transpose(
                    out=vg[:, 0:HD],
                    in_=v2[sub % 2][:, (s % 8) * 8 + 4 * half:
                                    (s % 8) * 8 + 4 * half + 4, :]
                    .rearrange("p t g -> p (t g)"))
            state["gi"] = gi + 1
            state["prev"] = (gi, s)

        def phase_b1(c, prev):
            gi, s = prev
            av = psM.tile([P, HD + 1], f32, tag="m", name="av")
            bd = bds[gi % NBD]
            nc.tensor.matmul(av, bd[:, 0:P], vgs[(2 * gi) % NVG],
                             start=True, stop=False)
            nc.tensor.matmul(av, bd[:, 136:136 + P],
                             vgs[(2 * gi + 1) % NVG],
                             start=False, stop=True)
            invz = ivp.tile([P, 1], f32, tag="iv", name="invz")
            nc.vector.reciprocal(invz, av[:, HD:HD + 1])
            ao = aop.tile([P, P], bf16, tag="ao", name="ao")
            nc.vector.tensor_scalar_mul(ao, av[:, 0:HD], invz)
            return ao

        def phase_b2(c, prev, ao):
            gi, s = prev
            aops = psM.tile([P, P], bf16, tag="m", name="aops")
            nc.tensor.transpose(aops, ao, ident)
            nc.scalar.activation(
                out=aoT[c % 2][:, :, 8 * s:8 * s + 8],
                in_=aops.rearrange("p (h t) -> p h t", t=8),
                func=mybir.ActivationFunctionType.Copy)

        # ---------------- schedule ----------------
        load_x(0)
        load_x(1)
        for oi in range(NO):
            for m in range(3):
                pend.append(qkv_unit(0, oi, m))
        pump_all()

        for c in range(NCH):
            if c + 2 < NCH:
                load_x(c + 2)
            if c >= 1:
                for oi in range(NO):
                    pend.append(wo_unit(c - 1, oi))
            if c + 1 < NCH:
                for oi in range(NO):
                    for m in range(3):
                        pend.append(qkv_unit(c + 1, oi, m))
            relayout(c, 0, 0)
            relayout(c, 0, 1)
            state["prev"] = None
            for s in range(NG):
                prev = state["prev"]
                phase_a(c, s)
                pump(6)
                if prev is not None:
                    ao = phase_b1(c, prev)
                    pump(5)
                    phase_b2(c, prev, ao)
                else:
                    pump(5)
                pump(6)
            prev = state["prev"]
            ao = phase_b1(c, prev)
            phase_b2(c, prev, ao)
        for oi in range(NO):
            pend.append(wo_unit(NCH - 1, oi))
        pump_all()

    nc.compile()
    return nc


def _get_program():
    if "nc" not in _cached:
        _cached["nc"] = _build_program()
    return _cached["nc"]


def kernel(x, Wq, Wk, Wv, Wo):
    import ml_dtypes
    from concourse.bass_utils import run_bass_kernel_spmd

    bfd = ml_dtypes.bfloat16
    B, S, H = x.shape
    assert (B * S, H) == (NCORES * TPC, E)
    nc = _get_program()

    def pretile(W):
        # [oi, p, e, o] with row oi*128+p, col e*128+o ; W is [out, in]
        A = np.asarray(W).reshape(NO, P, NE, P).transpose(0, 3, 2, 1)
        return np.ascontiguousarray(A.reshape(E, E).astype(bfd))

    Wqp = pretile(Wq)
    Wkp = pretile(Wk)
    Wvp = pretile(Wv)
    Wop = pretile(Wo)

    xf = np.asarray(x).reshape(B * S, H)
    in_maps = []
    for i in range(NCORES):
        xT = np.ascontiguousarray(
            xf[i * TPC:(i + 1) * TPC, :].T.astype(bfd))
        in_maps.append({"xT": xT, "Wq": Wqp, "Wk": Wkp,
                       "Wv": Wvp, "Wo": Wop})

    import os
    trace = bool(int(os.environ.get("BASS_KERNEL_TRACE", "0")))
    res = run_bass_kernel_spmd(nc, in_maps, core_ids=list(range(NCORES)),
                               trace=trace)
    if trace:
        _cached["last_results"] = res
    parts = [res.results[i]["yT"].T for i in range(NCORES)]
    y = np.concatenate(parts, axis=0).reshape(B, S, H)
    return np.ascontiguousarray(y.astype(np.float32))


# revision 20
# speedup vs baseline: 1.7527x; 1.7527x over previous
"""Trainium2 Bass kernel for per-token multi-head self-attention.

Computation (per token t):
  q,k,v = x @ W{q,k,v}.T ; scores = (q_t k_t^T)/sqrt(128) over heads [16x16]
  out_t = softmax(scores) @ v_t ; y = out @ Wo.T

Sharding: data-parallel over the 16384 tokens -> 8 cores x 2048 tokens.

Fully-fused single-pass structure, all matmul operands in bf16 (fp32 PSUM
accumulation; CPU-simulated pipeline rel err ~4.4e-3 vs the 2e-2 gate):
  - One stream of "GEMM units" (QKV projection tiles and Wo output tiles)
    is interleaved ("pumped") between the small attention-middle ops so the
    PE never idles on the middle's cross-engine dependency chains.
  - qkv never round-trips through DRAM: QKV units for chunk c+1 run (as
    pump filler) during the attention middle of chunk c, writing SBUF
    double buffers.
  - Attention middle processes 8 tokens per group: per-token 16x16 score
    matmuls (4 PE column-groups x 2 rounds) -> one exp ACT -> block-diag
    [128,128] attn matrix (copies split across gpsimd/vector/scalar) ->
    one AV matmul against the PE-transposed V block with a ones column
    producing the softmax normalizer -> per-partition 1/Z scale -> one
    PE-transpose back to feature-major layout for the Wo GEMM.
Weights are host-side pre-tiled so every weight-tile DMA reads 2-4KB
contiguous runs.
"""
import math
from contextlib import ExitStack

import numpy as np

NCORES = 8
E = 2048          # hidden
NH = 16           # heads
HD = 128          # head dim
TPC = 2048        # tokens per core
TC = 512          # token chunk
P = 128
NE = E // P       # 16 contraction tiles
NO = E // P       # 16 output tiles
NCH = TPC // TC   # 4 chunks
NG = TC // 8      # 64 8-token groups per chunk
SC = 1.0 / math.sqrt(HD)

_cached = {}


_dbg = {}


def _build_program():
    import concourse.bass as bass
    import concourse.tile as tile
    from concourse import bacc, mybir
    from concourse.masks import make_identity

    f32 = mybir.dt.float32
    bf16 = mybir.dt.bfloat16

    nc = bacc.Bacc("TRN2", target_bir_lowering=False, debug=False)

    xT_d = nc.dram_tensor("xT", [E, TPC], bf16, kind="ExternalInput").ap()
    # pre-tiled weights: row oi*128+p, col e*128+o  (p = input-feature within
    # e-slice for QKV; for Wo: p = head-dim within head h, col h*128+o)
    Wq_d = nc.dram_tensor("Wq", [E, E], bf16, kind="ExternalInput").ap()
    Wk_d = nc.dram_tensor("Wk", [E, E], bf16, kind="ExternalInput").ap()
    Wv_d = nc.dram_tensor("Wv", [E, E], bf16, kind="ExternalInput").ap()
    Wo_d = nc.dram_tensor("Wo", [E, E], bf16, kind="ExternalInput").ap()
    yT_d = nc.dram_tensor("yT", [E, TPC], f32, kind="ExternalOutput").ap()

    with tile.TileContext(nc) as tc, ExitStack() as ctx:
        glob = ctx.enter_context(tc.tile_pool(name="glob", bufs=1))
        ident = glob.tile([P, P], bf16)
        make_identity(nc, ident)

        xp = ctx.enter_context(tc.tile_pool(name="xp", bufs=1))
        qkvp = ctx.enter_context(tc.tile_pool(name="qkvp", bufs=1))
        aotp = ctx.enter_context(tc.tile_pool(name="aotp", bufs=1))
        v2p = ctx.enter_context(tc.tile_pool(name="v2p", bufs=1))
        bdp = ctx.enter_context(tc.tile_pool(name="bdp", bufs=1))
        vgp = ctx.enter_context(tc.tile_pool(name="vgp", bufs=1))
        wp = ctx.enter_context(tc.tile_pool(name="wp", bufs=4))
        esp = ctx.enter_context(tc.tile_pool(name="esp", bufs=3))
        aop = ctx.enter_context(tc.tile_pool(name="aop", bufs=3))
        ivp = ctx.enter_context(tc.tile_pool(name="ivp", bufs=3))
        ysp = ctx.enter_context(tc.tile_pool(name="ysp", bufs=3))
        psG = ctx.enter_context(tc.tile_pool(name="psG", bufs=3, space="PSUM"))
        psS = ctx.enter_context(tc.tile_pool(name="psS", bufs=1, space="PSUM"))
        psM = ctx.enter_context(tc.tile_pool(name="psM", bufs=4, space="PSUM"))

        # persistent double buffers
        xb = [xp.tile([P, NE, TC], bf16, tag=f"x{i}", name=f"x{i}")
              for i in range(2)]
        qkv = [[qkvp.tile([P, NO, TC], bf16, tag=f"qkv{m}_{i}",
                          name=f"qkv{m}_{i}")
                for i in range(2)] for m in range(3)]
        aoT = [aotp.tile([P, NH, TC], bf16, tag=f"aoT{i}", name=f"aoT{i}")
               for i in range(2)]
        v2 = []
        for i in range(2):
            t = v2p.tile([P, 64, 32], bf16, tag=f"v2_{i}", name=f"v2_{i}")
            nc.vector.memset(t, 0.0)
            v2.append(t)
        NBD = 4
        bds = []
        for i in range(NBD):
            t = bdp.tile([P, 280], bf16, tag=f"bd{i}", name=f"bd{i}")
            nc.vector.memset(t, 0.0)
            bds.append(t)
        NVG = 8
        vgs = []
        for i in range(NVG):
            t = vgp.tile([P, HD + 1], bf16, tag=f"vg{i}", name=f"vg{i}")
            nc.vector.memset(t, 0.0)
            nc.vector.memset(t[:, HD:HD + 1], 1.0)
            vgs.append(t)

        wmats = [Wq_d, Wk_d, Wv_d]

        def load_x(c):
            for e in range(NE):
                nc.sync.dma_start(
                    out=xb[c % 2][:, e, :],
                    in_=xT_d[e * P:(e + 1) * P, c * TC:(c + 1) * TC])

        # ---------------- GEMM unit machinery ----------------
        # Each unit: (prefetch_fn -> returns w tile, gen_fn(w) yields per MM)
        def qkv_unit(c, oi, m):
            def pre():
                wt = wp.tile([P, NE, P], bf16, tag="w", name="w")
                wf = wt.rearrange("p e o -> p (e o)")
                nc.sync.dma_start(out=wf[:, 0:E // 2],
                                  in_=wmats[m][oi * P:(oi + 1) * P, 0:E // 2])
                nc.sync.dma_start(out=wf[:, E // 2:E],
                                  in_=wmats[m][oi * P:(oi + 1) * P, E // 2:E])
                return wt

            def gen(wt):
                acc = psG.tile([P, TC], f32, tag="acc", name="acc")
                for e in range(NE):
                    nc.tensor.matmul(acc, wt[:, e, :], xb[c % 2][:, e, :],
                                     start=(e == 0), stop=(e == NE - 1))
                    yield
                nc.scalar.activation(
                    out=qkv[m][c % 2][:, oi, :], in_=acc,
                    func=mybir.ActivationFunctionType.Copy)

            return pre, gen

        def wo_unit(c, oi):
            def pre():
                wt = wp.tile([P, NH, P], bf16, tag="w", name="w")
                wf = wt.rearrange("p h o -> p (h o)")
                nc.sync.dma_start(out=wf[:, 0:E // 2],
                                  in_=Wo_d[oi * P:(oi + 1) * P, 0:E // 2])
                nc.sync.dma_start(out=wf[:, E // 2:E],
                                  in_=Wo_d[oi * P:(oi + 1) * P, E // 2:E])
                return wt

            def gen(wt):
                yp = psG.tile([P, TC], f32, tag="acc", name="yp")
                for h in range(NH):
                    nc.tensor.matmul(yp, wt[:, h, :], aoT[c % 2][:, h, :],
                                     start=(h == 0), stop=(h == NH - 1))
                    yield
                ys = ysp.tile([P, TC], f32, tag="ys", name="ys")
                nc.vector.tensor_copy(ys, yp)
                nc.sync.dma_start(
                    out=yT_d[oi * P:(oi + 1) * P, c * TC:(c + 1) * TC],
                    in_=ys)

            return pre, gen

        pend = []          # [pre, gen] not yet prefetched
        active = []        # generators with w already fetched
        LOOKAHEAD = 4

        def refill():
            while pend and len(active) < LOOKAHEAD:
                pre, gen = pend.pop(0)
                active.append(gen(pre()))

        def pump(n):
            refill()
            while n > 0 and active:
                g = active[0]
                try:
                    next(g)
                    n -= 1
                except StopIteration:
                    active.pop(0)
                    refill()

        def pump_all():
            refill()
            while active:
                try:
                    next(active[0])
                except StopIteration:
                    active.pop(0)
                    refill()

        # ---------------- attention middle ----------------
        def relayout(c, sub, half):
            t0 = sub * 64 + 32 * half
            nc.gpsimd.tensor_copy(
                v2[sub % 2][:, 32 * half:32 * (half + 1), 0:NH],
                qkv[2][c % 2][:, :, t0:t0 + 32]
                .rearrange("p g t -> p t g"))

        state = {"gi": 0, "prev": None}

        def phase_a(c, s):
            sub = s // 8
            if s % 8 == 2 and sub + 1 < 8:
                relayout(c, sub + 1, 0)
            if s % 8 == 5 and sub + 1 < 8:
                relayout(c, sub + 1, 1)
            q_sb, k_sb = qkv[0][c % 2], qkv[1][c % 2]
            sc = psS.tile([P, 32], f32, tag="sc", name="sc")
            t0 = s * 8
            for j in range(4):
                for half in range(2):
                    t = t0 + 4 * half + j
                    nc.tensor.matmul(
                        sc[32 * j:32 * j + NH, 16 * half:16 * half + 16],
                        k_sb[:, :, t], q_sb[:, :, t],
                        start=True, stop=True,
                        tile_position=(0, 32 * j))
            es = esp.tile([P, 32], bf16, tag="es", name="es")
            nc.scalar.activation(out=es, in_=sc,
                                 func=mybir.ActivationFunctionType.Exp,
                                 scale=SC)
            gi = state["gi"]
            # one wide block-diag tile: col 140*hf + 8h + j, rows 32j+g
            bd = bds[gi % NBD]
            bdv = bd.rearrange("p (hf q) -> p hf q", hf=2)
            esv = es.rearrange("p (hf g) -> p hf g", hf=2)
            for j in range(4):
                dst = bdv[32 * j:32 * j + 16, :, j:j + 121:8]
                srcv = esv[32 * j:32 * j + 16, :, :]
                if j in (0, 3):
                    nc.gpsimd.tensor_copy(dst, srcv)
                elif j == 1:
                    nc.scalar.activation(
                        out=dst, in_=srcv,
                        func=mybir.ActivationFunctionType.Copy)
                else:
                    nc.vector.tensor_copy(dst, srcv)
            for half in range(2):
                # V block transpose: [128 d, 4t*32] -> [(32t+g), d]
                vps = psM.tile([P, P], bf16, tag="m", name="vps")
                nc.tensor.transpose(
                    vps,
                    v2[sub % 2][:, (s % 8) * 8 + 4 * half:
                                (s % 8) * 8 + 4 * half + 4, :]
                    .rearrange("p t g -> p (t g)"),
                    ident)
                vg = vgs[(2 * gi + half) % NVG]
                if half == 0:
                    nc.vector.tensor_copy(vg[:, 0:HD], vps)
                else:
                    nc.scalar.activation(
                        out=vg[:, 0:HD], in_=vps,
                        func=mybir.ActivationFunctionType.Copy)
            state["gi"] = gi + 1
            state["prev"] = (gi, s)

        def phase_b1(c, prev):
            gi, s = prev
            av = psM.tile([P, HD + 1], f32, tag="m", name="av")
            bd = bds[gi % NBD]
            nc.tensor.matmul(av, bd[:, 0:P], vgs[(2 * gi) % NVG],
                             start=True, stop=False)
            nc.tensor.matmul(av, bd[:, 136:136 + P],
                             vgs[(2 * gi + 1) % NVG],
                             start=False, stop=True)
            invz = ivp.tile([P, 1], f32, tag="iv", name="invz")
            nc.vector.reciprocal(invz, av[:, HD:HD + 1])
            ao = aop.tile([P, P], bf16, tag="ao", name="ao")
            nc.vector.tensor_scalar_mul(ao, av[:, 0:HD], invz)
            return ao

        def phase_b2(c, prev, ao):
            gi, s = prev
            aops = psM.tile([P, P], bf16, tag="m", name="aops")
            nc.tensor.transpose(aops, ao, ident)
            nc.scalar.activation(
                out=aoT[c % 2][:, :, 8 * s:8 * s + 8],
                in_=aops.rearrange("p (h t) -> p h t", t=8),
                func=mybir.ActivationFunctionType.Copy)

        # ---------------- schedule ----------------
        load_x(0)
        load_x(1)
        for oi in range(NO):
            for m in range(3):
                pend.append(qkv_unit(0, oi, m))
        pump_all()

        for c in range(NCH):
            if c + 2 < NCH:
                load_x(c + 2)
            if c >= 1:
                for oi in range(NO):
                    pend.append(wo_unit(c - 1, oi))
            if c + 1 < NCH:
                for oi in range(NO):
                    for m in range(3):
                        pend.append(qkv_unit(c + 1, oi, m))
            relayout(c, 0, 0)
            relayout(c, 0, 1)
            state["prev"] = None
            for s in range(NG):
                prev = state["prev"]
                phase_a(c, s)
                pump(6)
                if prev is not None:
                    ao = phase_b1(c, prev)
                    pump(5)
                    phase_b2(c, prev, ao)
                else:
                    pump(5)
                pump(6)
            prev = state["prev"]
            ao = phase_b1(c, prev)
            phase_b2(c, prev, ao)
        for oi in range(NO):
            pend.append(wo_unit(NCH - 1, oi))
        pump_all()

    nc.compile()
    return nc


def _get_program():
    if "nc" not in _cached:
        _cached["nc"] = _build_program()
    return _cached["nc"]


def kernel(x, Wq, Wk, Wv, Wo):
    import ml_dtypes
    from concourse.bass_utils import run_bass_kernel_spmd

    bfd = ml_dtypes.bfloat16
    B, S, H = x.shape
    assert (B * S, H) == (NCORES * TPC, E)
    nc = _get_program()

    def pretile(W):
        # [oi, p, e, o] with row oi*128+p, col e*128+o ; W is [out, in]
        A = np.asarray(W).reshape(NO, P, NE, P).transpose(0, 3, 2, 1)
        return np.ascontiguousarray(A.reshape(E, E).astype(bfd))

    Wqp = pretile(Wq)
    Wkp = pretile(Wk)
    Wvp = pretile(Wv)
    Wop = pretile(Wo)

    xf = np.asarray(x).reshape(B * S, H)
    in_maps = []
    for i in range(NCORES):
        xT = np.ascontiguousarray(
            xf[i * TPC:(i + 1) * TPC, :].T.astype(bfd))
        in_maps.append({"xT": xT, "Wq": Wqp, "Wk": Wkp,
                       "Wv": Wvp, "Wo": Wop})

    import os
    trace = bool(int(os.environ.get("BASS_KERNEL_TRACE", "0")))
    res = run_bass_kernel_spmd(nc, in_maps, core_ids=list(range(NCORES)),
                               trace=trace)
    if trace:
        _cached["last_results"] = res
    parts = [res.results[i]["yT"].T for i in range(NCORES)]
    y = np.concatenate(parts, axis=0).reshape(B, S, H)
    return np.ascontiguousarray(y.astype(np.float32))


# revision 21
# speedup vs baseline: 1.8276x; 1.0427x over previous
"""Trainium2 Bass kernel for per-token multi-head self-attention.

Computation (per token t):
  q,k,v = x @ W{q,k,v}.T ; scores = (q_t k_t^T)/sqrt(128) over heads [16x16]
  out_t = softmax(scores) @ v_t ; y = out @ Wo.T

Sharding: data-parallel over the 16384 tokens -> 8 cores x 2048 tokens.

Fully-fused single-pass structure, all matmul operands in bf16 (fp32 PSUM
accumulation; CPU-simulated pipeline rel err ~4.4e-3 vs the 2e-2 gate):
  - One stream of "GEMM units" (QKV projection tiles and Wo output tiles)
    is interleaved ("pumped") between the small attention-middle ops so the
    PE never idles on the middle's cross-engine dependency chains.
  - qkv never round-trips through DRAM: QKV units for chunk c+1 run (as
    pump filler) during the attention middle of chunk c, writing SBUF
    double buffers.
  - Attention middle processes 8 tokens per group: per-token 16x16 score
    matmuls (4 PE column-groups x 2 rounds) -> one exp ACT -> block-diag
    [128,128] attn matrix (copies split across gpsimd/vector/scalar) ->
    one AV matmul against the PE-transposed V block with a ones column
    producing the softmax normalizer -> per-partition 1/Z scale -> one
    PE-transpose back to feature-major layout for the Wo GEMM.
Weights are host-side pre-tiled so every weight-tile DMA reads 2-4KB
contiguous runs.
"""
import math
from contextlib import ExitStack

import numpy as np

NCORES = 8
E = 2048          # hidden
NH = 16           # heads
HD = 128          # head dim
TPC = 2048        # tokens per core
TC = 512          # token chunk
P = 128
NE = E // P       # 16 contraction tiles
NO = E // P       # 16 output tiles
NCH = TPC // TC   # 4 chunks
NG = TC // 8      # 64 8-token groups per chunk
SC = 1.0 / math.sqrt(HD)

_cached = {}


_dbg = {}


def _build_program():
    import concourse.bass as bass
    import concourse.tile as tile
    from concourse import bacc, mybir
    from concourse.masks import make_identity

    f32 = mybir.dt.float32
    bf16 = mybir.dt.bfloat16

    nc = bacc.Bacc("TRN2", target_bir_lowering=False, debug=False)

    xT_d = nc.dram_tensor("xT", [E, TPC], bf16, kind="ExternalInput").ap()
    # pre-tiled weights: row oi*128+p, col e*128+o  (p = input-feature within
    # e-slice for QKV; for Wo: p = head-dim within head h, col h*128+o)
    Wq_d = nc.dram_tensor("Wq", [E, E], bf16, kind="ExternalInput").ap()
    Wk_d = nc.dram_tensor("Wk", [E, E], bf16, kind="ExternalInput").ap()
    Wv_d = nc.dram_tensor("Wv", [E, E], bf16, kind="ExternalInput").ap()
    Wo_d = nc.dram_tensor("Wo", [E, E], bf16, kind="ExternalInput").ap()
    yT_d = nc.dram_tensor("yT", [E, TPC], f32, kind="ExternalOutput").ap()

    with tile.TileContext(nc) as tc, ExitStack() as ctx:
        glob = ctx.enter_context(tc.tile_pool(name="glob", bufs=1))
        ident = glob.tile([P, P], bf16)
        make_identity(nc, ident)

        xp = ctx.enter_context(tc.tile_pool(name="xp", bufs=1))
        qkvp = ctx.enter_context(tc.tile_pool(name="qkvp", bufs=1))
        aotp = ctx.enter_context(tc.tile_pool(name="aotp", bufs=1))
        v2p = ctx.enter_context(tc.tile_pool(name="v2p", bufs=1))
        bdp = ctx.enter_context(tc.tile_pool(name="bdp", bufs=1))
        vgp = ctx.enter_context(tc.tile_pool(name="vgp", bufs=1))
        wp = ctx.enter_context(tc.tile_pool(name="wp", bufs=4))
        esp = ctx.enter_context(tc.tile_pool(name="esp", bufs=3))
        aop = ctx.enter_context(tc.tile_pool(name="aop", bufs=3))
        ivp = ctx.enter_context(tc.tile_pool(name="ivp", bufs=3))
        ysp = ctx.enter_context(tc.tile_pool(name="ysp", bufs=3))
        psG = ctx.enter_context(tc.tile_pool(name="psG", bufs=3, space="PSUM"))
        psS = ctx.enter_context(tc.tile_pool(name="psS", bufs=1, space="PSUM"))
        psM = ctx.enter_context(tc.tile_pool(name="psM", bufs=4, space="PSUM"))

        # persistent double buffers
        xb = [xp.tile([P, NE, TC], bf16, tag=f"x{i}", name=f"x{i}")
              for i in range(2)]
        qkv = [[qkvp.tile([P, NO, TC], bf16, tag=f"qkv{m}_{i}",
                          name=f"qkv{m}_{i}")
                for i in range(2)] for m in range(3)]
        aoT = [aotp.tile([P, NH, TC], bf16, tag=f"aoT{i}", name=f"aoT{i}")
               for i in range(2)]
        v2 = []
        for i in range(2):
            t = v2p.tile([P, 64, 32], bf16, tag=f"v2_{i}", name=f"v2_{i}")
            nc.vector.memset(t, 0.0)
            v2.append(t)
        NBD = 4
        bds = []
        for i in range(NBD):
            t = bdp.tile([P, 280], bf16, tag=f"bd{i}", name=f"bd{i}")
            nc.vector.memset(t, 0.0)
            bds.append(t)
        NVG = 8
        vgs = []
        for i in range(NVG):
            t = vgp.tile([P, HD + 1], bf16, tag=f"vg{i}", name=f"vg{i}")
            nc.vector.memset(t, 0.0)
            nc.vector.memset(t[:, HD:HD + 1], 1.0)
            vgs.append(t)

        wmats = [Wq_d, Wk_d, Wv_d]

        def load_x(c):
            for e in range(NE):
                nc.sync.dma_start(
                    out=xb[c % 2][:, e, :],
                    in_=xT_d[e * P:(e + 1) * P, c * TC:(c + 1) * TC])

        # ---------------- GEMM unit machinery ----------------
        # Each unit: (prefetch_fn -> returns w tile, gen_fn(w) yields per MM)
        def qkv_unit(c, oi, m):
            def pre():
                wt = wp.tile([P, NE, P], bf16, tag="w", name="w")
                wf = wt.rearrange("p e o -> p (e o)")
                nc.sync.dma_start(out=wf[:, 0:E // 2],
                                  in_=wmats[m][oi * P:(oi + 1) * P, 0:E // 2])
                nc.sync.dma_start(out=wf[:, E // 2:E],
                                  in_=wmats[m][oi * P:(oi + 1) * P, E // 2:E])
                return wt

            def gen(wt):
                acc = psG.tile([P, TC], f32, tag="acc", name="acc")
                for e in range(NE):
                    nc.tensor.matmul(acc, wt[:, e, :], xb[c % 2][:, e, :],
                                     start=(e == 0), stop=(e == NE - 1))
                    yield
                nc.scalar.activation(
                    out=qkv[m][c % 2][:, oi, :], in_=acc,
                    func=mybir.ActivationFunctionType.Copy)

            return pre, gen

        def wo_unit(c, oi):
            def pre():
                wt = wp.tile([P, NH, P], bf16, tag="w", name="w")
                wf = wt.rearrange("p h o -> p (h o)")
                nc.sync.dma_start(out=wf[:, 0:E // 2],
                                  in_=Wo_d[oi * P:(oi + 1) * P, 0:E // 2])
                nc.sync.dma_start(out=wf[:, E // 2:E],
                                  in_=Wo_d[oi * P:(oi + 1) * P, E // 2:E])
                return wt

            def gen(wt):
                yp = psG.tile([P, TC], f32, tag="acc", name="yp")
                for h in range(NH):
                    nc.tensor.matmul(yp, wt[:, h, :], aoT[c % 2][:, h, :],
                                     start=(h == 0), stop=(h == NH - 1))
                    yield
                ys = ysp.tile([P, TC], f32, tag="ys", name="ys")
                nc.vector.tensor_copy(ys, yp)
                nc.sync.dma_start(
                    out=yT_d[oi * P:(oi + 1) * P, c * TC:(c + 1) * TC],
                    in_=ys)

            return pre, gen

        pend = []          # [pre, gen] not yet prefetched
        active = []        # generators with w already fetched
        LOOKAHEAD = 4

        def refill():
            while pend and len(active) < LOOKAHEAD:
                pre, gen = pend.pop(0)
                active.append(gen(pre()))

        def pump(n):
            refill()
            while n > 0 and active:
                g = active[0]
                try:
                    next(g)
                    n -= 1
                except StopIteration:
                    active.pop(0)
                    refill()

        def pump_all():
            refill()
            while active:
                try:
                    next(active[0])
                except StopIteration:
                    active.pop(0)
                    refill()

        # ---------------- attention middle ----------------
        def relayout(c, sub, half):
            t0 = sub * 64 + 32 * half
            nc.gpsimd.tensor_copy(
                v2[sub % 2][:, 32 * half:32 * (half + 1), 0:NH],
                qkv[2][c % 2][:, :, t0:t0 + 32]
                .rearrange("p g t -> p t g"))

        state = {"gi": 0, "prev": None}

        def phase_a(c, s):
            sub = s // 8
            if s % 8 == 2 and sub + 1 < 8:
                relayout(c, sub + 1, 0)
            if s % 8 == 5 and sub + 1 < 8:
                relayout(c, sub + 1, 1)
            q_sb, k_sb = qkv[0][c % 2], qkv[1][c % 2]
            sc = psS.tile([P, 32], f32, tag="sc", name="sc")
            t0 = s * 8
            for j in range(4):
                for half in range(2):
                    t = t0 + 4 * half + j
                    nc.tensor.matmul(
                        sc[32 * j:32 * j + NH, 16 * half:16 * half + 16],
                        k_sb[:, :, t], q_sb[:, :, t],
                        start=True, stop=True,
                        tile_position=(0, 32 * j))
            es = esp.tile([P, 32], bf16, tag="es", name="es")
            nc.scalar.activation(out=es, in_=sc,
                                 func=mybir.ActivationFunctionType.Exp,
                                 scale=SC)
            gi = state["gi"]
            # one wide block-diag tile: col 140*hf + 8h + j, rows 32j+g
            bd = bds[gi % NBD]
            bdv = bd.rearrange("p (hf q) -> p hf q", hf=2)
            esv = es.rearrange("p (hf g) -> p hf g", hf=2)
            for j in range(4):
                dst = bdv[32 * j:32 * j + 16, :, j:j + 121:8]
                srcv = esv[32 * j:32 * j + 16, :, :]
                if j in (0, 3):
                    nc.gpsimd.tensor_copy(dst, srcv)
                elif j == 1:
                    nc.scalar.activation(
                        out=dst, in_=srcv,
                        func=mybir.ActivationFunctionType.Copy)
                else:
                    nc.vector.tensor_copy(dst, srcv)
            for half in range(2):
                # V block transpose: [128 d, 4t*32] -> [(32t+g), d]
                vps = psM.tile([P, P], bf16, tag="m", name="vps")
                nc.tensor.transpose(
                    vps,
                    v2[sub % 2][:, (s % 8) * 8 + 4 * half:
                                (s % 8) * 8 + 4 * half + 4, :]
                    .rearrange("p t g -> p (t g)"),
                    ident)
                vg = vgs[(2 * gi + half) % NVG]
                if half == 0:
                    nc.vector.tensor_copy(vg[:, 0:HD], vps)
                else:
                    nc.scalar.activation(
                        out=vg[:, 0:HD], in_=vps,
                        func=mybir.ActivationFunctionType.Copy)
            state["gi"] = gi + 1
            state["prev"] = (gi, s)

        def phase_b1(c, prev):
            gi, s = prev
            av = psM.tile([P, HD + 1], f32, tag="m", name="av")
            bd = bds[gi % NBD]
            nc.tensor.matmul(av, bd[:, 0:P], vgs[(2 * gi) % NVG],
                             start=True, stop=False)
            nc.tensor.matmul(av, bd[:, 136:136 + P],
                             vgs[(2 * gi + 1) % NVG],
                             start=False, stop=True)
            invz = ivp.tile([P, 1], f32, tag="iv", name="invz")
            nc.vector.reciprocal(invz, av[:, HD:HD + 1])
            ao = aop.tile([P, P], bf16, tag="ao", name="ao")
            nc.vector.tensor_scalar_mul(ao, av[:, 0:HD], invz)
            return ao

        def phase_b2(c, prev, ao):
            gi, s = prev
            aops = psM.tile([P, P], bf16, tag="m", name="aops")
            nc.tensor.transpose(aops, ao, ident)
            nc.scalar.activation(
                out=aoT[c % 2][:, :, 8 * s:8 * s + 8],
                in_=aops.rearrange("p (h t) -> p h t", t=8),
                func=mybir.ActivationFunctionType.Copy)

        # ---------------- schedule ----------------
        load_x(0)
        load_x(1)
        for oi in range(NO):
            for m in range(3):
                pend.append(qkv_unit(0, oi, m))
        pump_all()

        for c in range(NCH):
            if c + 2 < NCH:
                load_x(c + 2)
            if c >= 1:
                for oi in range(NO):
                    pend.append(wo_unit(c - 1, oi))
            if c + 1 < NCH:
                for oi in range(NO):
                    for m in range(3):
                        pend.append(qkv_unit(c + 1, oi, m))
            relayout(c, 0, 0)
            relayout(c, 0, 1)
            state["prev"] = None
            # pace filler evenly across the chunk's slots: chunk 0 has 768
            # MMs (QKV only), 1-2 have 1024, 3 has 256 (Wo only, no drain
            # deadline) -- draining early starves the tail slots.
            fa, fb1, fb2 = {0: (5, 4, 4), 1: (6, 5, 6),
                            2: (6, 5, 6), 3: (2, 1, 1)}[c]
            for s in range(NG):
                prev = state["prev"]
                phase_a(c, s)
                pump(fa)
                if prev is not None:
                    ao = phase_b1(c, prev)
                    pump(fb1)
                    phase_b2(c, prev, ao)
                else:
                    pump(fb1)
                pump(fb2)
            prev = state["prev"]
            ao = phase_b1(c, prev)
            phase_b2(c, prev, ao)
        for oi in range(NO):
            pend.append(wo_unit(NCH - 1, oi))
        pump_all()

    nc.compile()
    return nc


def _get_program():
    if "nc" not in _cached:
        _cached["nc"] = _build_program()
    return _cached["nc"]


def kernel(x, Wq, Wk, Wv, Wo):
    import ml_dtypes
    from concourse.bass_utils import run_bass_kernel_spmd

    bfd = ml_dtypes.bfloat16
    B, S, H = x.shape
    assert (B * S, H) == (NCORES * TPC, E)
    nc = _get_program()

    def pretile(W):
        # [oi, p, e, o] with row oi*128+p, col e*128+o ; W is [out, in]
        A = np.asarray(W).reshape(NO, P, NE, P).transpose(0, 3, 2, 1)
        return np.ascontiguousarray(A.reshape(E, E).astype(bfd))

    Wqp = pretile(Wq)
    Wkp = pretile(Wk)
    Wvp = pretile(Wv)
    Wop = pretile(Wo)

    xf = np.asarray(x).reshape(B * S, H)
    in_maps = []
    for i in range(NCORES):
        xT = np.ascontiguousarray(
            xf[i * TPC:(i + 1) * TPC, :].T.astype(bfd))
        in_maps.append({"xT": xT, "Wq": Wqp, "Wk": Wkp,
                       "Wv": Wvp, "Wo": Wop})

    import os
    trace = bool(int(os.environ.get("BASS_KERNEL_TRACE", "0")))
    res = run_bass_kernel_spmd(nc, in_maps, core_ids=list(range(NCORES)),
                               trace=trace)
    if trace:
        _cached["last_results"] = res
    parts = [res.results[i]["yT"].T for i in range(NCORES)]
    y = np.concatenate(parts, axis=0).reshape(B, S, H)
    return np.ascontiguousarray(y.astype(np.float32))
